# revision 39
# baseline (speedup 1.0000x reference)
"""DenseCRFLoss on 8 Trainium2 NeuronCores.

Math: loss = -W/N * sum_k s_k^T K s_k per image, K[p,q] = exp(-0.5*||f_p-f_q||^2),
f = (x/50, y/50, rgb/15) on the 64x64 downsampled image, P=4096 pixels.

Device strategy (per core, SPMD-uniform program; data differs per core):
  - G[p,q] = log-kernel exponent is produced by ONE fp8e4 DoubleRow matmul
    per 512-col PSUM bank (0.5 PE cycles/column): features are multi-term
    fp8-split (4 terms rgb / 2 terms xy, 10/3 cross products kept) and the
    -|f|^2/2 row/col terms enter as 2-term fp8 rows whose residual is folded
    into the segmentation weights on the host. 42 contraction rows,
    pair-packed into 21 partitions x 2 (DoubleRow layout).
  - exp runs on TWO engines in parallel (GPSIMD cannot access PSUM, and DVE
    has no native transcendentals): ACT banks use activation Exp with
    scale=64*ALPHA*ln2; DVE banks use two custom-DVE ops registered at build
    time: EXP2P_ANT computes q = ((d z + c) z + c) z + 1 on clamped
    z = max(G, ZLO) (the equal middle coefficients free a scalar slot; the
    NRT DVE executor rejects Src1 combined with imm2), and EXP2S_ANT squares
    six times, so q^64 = 2^(64*ALPHA*z) = K. Host pre-scales G by
    1/(64*ALPHA). Banks are assigned to engines by a load-balancing greedy.
  - The quadratic form uses "flipped" matmuls: E (bf16, SBUF) is the
    STATIONARY [128,128] and the per-block segmentation weights [128,2] are
    the moving side, so each 128-col chunk costs 2 PE cycles instead of 128+.
    Results accumulate over a quad's row-blocks in one shared PSUM bank
    (pending-zero first-touch gives each chunk a zeroed start). Host dots
    the [128, 18*4*2] result with column weights in fp64.
  - Triangle harvesting: each image's 8x8 chunk grid upper triangle = 36
    quads; 2 cores per image, 18 quads each, straddle (diagonal) quads at
    fixed slots. Diagonal 128x128 subtiles are computed UNMASKED but
    pre-halved via a -1/(64*ALPHA) bias contraction row in per-block
    duplicate moving stacks, so 2*sum(partials) = full quadratic form.
"""

import numpy as np
import ml_dtypes

WEIGHT = 2e-9
SIGMA_RGB = 15.0
SIGMA_XY = 100.0
SCALE = 0.5
LOG2E = float(np.log2(np.e))
LN2 = float(np.log(2.0))

NQ = 18
STRADDLE_SLOTS = (5, 9, 13, 17)   # processing order == slot order
KP = 21                           # contraction partitions (42 rows / 2)
BANK = 512                        # PSUM bank width (fp32 cols)
RING = 7                          # G ring depth in banks (bank 8 = ubank)

_bf16 = ml_dtypes.bfloat16
_fp8 = ml_dtypes.float8_e4m3

# (stat_term, mov_term) index pairs per dim class (1-based split terms)
_PAIRS_XY = [(1, 1), (1, 2), (2, 1)]
_PAIRS_RGB = [(1, 1), (1, 2), (2, 1), (2, 2), (1, 3), (3, 1), (1, 4), (4, 1),
              (2, 3), (3, 2)]

_PROGRAM_CACHE = {}

# custom DVE exp2: q(z) = 1 + q1 z + q2 z^2 + q3 z^3 ~ 2^z on [ZLO, ZHI],
# then E = q^64 = 2^(64 z). G is produced pre-scaled by 1/64 (host-side).
_ZLO = -0.47
_Q1 = 0.693000828189046
_Q2 = 0.23822779470972869
_Q3 = 0.047725511307453415
_EXP2_OPS = {}


def _register_dve_exp2():
    if _EXP2_OPS:
        return _EXP2_OPS
    from concourse import dve_ops
    from concourse.dve_spec import (Spec, Src0, Src1, C0, C1, C2, One, maxx,
                                    sq, lower, spec_leaves)
    from concourse.dve_uop import DveOpSpec
    from concourse.dve_ops import DveOp

    def reg(name, spec):
        if name in dve_ops._SUB_OPCODE_FOR_NAME:
            _EXP2_OPS[name] = next(o for o in dve_ops.OPS if o.name == name)
            return
        row = dve_ops._CUSTOM_DVE_ROW_BASE + len(dve_ops.OPS)
        dve_ops._SUB_OPCODE_FOR_NAME[name] = row
        shas = {}
        for ver in ("v3", "v4"):
            s = DveOpSpec(name=name, opcode=row, uops=lower(spec, ver=ver),
                          rd1_en=Src1 in spec_leaves(spec))
            shas[ver] = s.sha(ver)
        op = DveOp(name, spec, subdim=False, uops_sha=shas)
        dve_ops.OPS.append(op)
        dve_ops.CUSTOM_DVE_SPECS[name] = op.spec
        _EXP2_OPS[name] = op

    _z = maxx(Src0, C0)
    _body = ((C1 * _z + C2) * _z + Src1) * _z + One

    def _ref_p(in0, in1, s0, s1, imm2):
        z = np.maximum(in0.astype(np.float32), np.float32(s0))
        return (((np.float32(s1) * z + np.float32(imm2)) * z + in1) * z
                + np.float32(1.0)).astype(np.float32)

    def _ref_s(in0, in1, s0, s1, imm2):
        x = in0.astype(np.float32)
        for _ in range(6):
            x = x * x
        return x

    reg("EXP2P_ANT", Spec(body=_body, reference=_ref_p))
    reg("EXP2S_ANT", Spec(body=sq(sq(sq(sq(sq(sq(Src0)))))), reference=_ref_s))
    return _EXP2_OPS


def _plan():
    """Pack G pieces into a stream of 512-col PSUM banks, then group
    consecutive banks into per-engine exp instructions (heterogeneous sizes,
    load-balanced greedily; groups never wrap the 7-bank ring).

    banks[b]: {pieces: [(slot, j, kind, src_lo, width, bank_off)],
               flips:  [(slot, j, chunk, bank_off)]}
    groups[k]: {eng, b0, n}  - exp instr over banks [b0, b0+n)
    kind: 1 = normal moving stack, 2 = diag (-1 bias) moving stack.
    """
    pieces = []
    for i in range(NQ):
        if i in STRADDLE_SLOTS:
            for j in range(4):
                pieces.append((i, j, 2, 128 * j, 128))
                if j < 3:
                    pieces.append((i, j, 1, 128 * j + 128, 512 - 128 * j - 128))
        else:
            for j in range(4):
                pieces.append((i, j, 1, 0, 512))

    banks = []
    cur = {"pieces": [], "flips": [], "fill": 0}
    for (i, j, kind, lo, w) in pieces:
        while w > 0:
            if cur["fill"] == BANK:
                banks.append(cur)
                cur = {"pieces": [], "flips": [], "fill": 0}
            take = min(w, BANK - cur["fill"])
            off = cur["fill"]
            cur["pieces"].append((i, j, kind, lo, take, off))
            for cc in range(lo // 128, (lo + take) // 128):
                cur["flips"].append((i, j, cc, off + (cc * 128 - lo)))
            cur["fill"] += take
            lo += take
            w -= take
    if cur["fill"]:
        banks.append(cur)
    nb = len(banks)

    # exp instr duration model (ns) for n banks
    # (GPSIMD cannot access PSUM on TRN2, so only ACT and DVE can exp)
    def dur(eng, n):
        cols = n * BANK
        if eng == "act":
            return cols * 0.8333 + 185 + 40
        return cols * 1.0417 + 125 + 45

    pref = {"act": 3, "dve": 1}
    t = {"act": 0.0, "dve": 900.0}   # DVE starts later (startup skew)
    groups = []
    b = 0
    while b < nb:
        eng = min(t, key=lambda e: t[e])
        n = min(pref[eng], nb - b)
        if nb - b <= 3:
            eng, n = "act", min(3, nb - b)   # short ACT drain at the tail
        t[eng] += dur(eng, n)
        groups.append({"eng": eng, "b0": b, "n": n})
        b += n
    return banks, groups


def _build_program(reps=1):
    import concourse.bacc as bacc
    import concourse.tile as tile
    from concourse import mybir

    nc = bacc.Bacc("TRN2", target_bir_lowering=False)
    dt = mybir.dt

    feat = nc.dram_tensor("feat", [KP, 2, 2, NQ + 4, 512], dt.float8e4,
                          kind="ExternalInput")
    aux = nc.dram_tensor("aux", [128, NQ * 8 + 1], dt.bfloat16,
                         kind="ExternalInput")
    out = nc.dram_tensor("out", [128, NQ * 8], dt.float32, kind="ExternalOutput")

    banks, groups = _plan()
    ng = len(groups)

    with tile.TileContext(nc) as tc:
        with (
            tc.tile_pool(name="consts", bufs=1) as consts,
            tc.tile_pool(name="gpa", bufs=3, space="PSUM") as gpa,
            tc.tile_pool(name="gpd", bufs=1, space="PSUM") as gpd,
            tc.tile_pool(name="gpp", bufs=2, space="PSUM") as gpp,
            tc.tile_pool(name="upool", bufs=1, space="PSUM") as upool,
            tc.tile_pool(name="epool", bufs=36) as epool,
        ):
            feat_sb = consts.tile([KP, 2, 2, NQ + 4, 512], dt.float8e4)
            aux_sb = consts.tile([128, NQ * 8 + 1], dt.bfloat16)
            warm = consts.tile([128, 1], dt.float32)

            # ACT table warm-up before any real dependency
            nc.vector.memset(warm, 0.0)
            nc.scalar.activation(out=warm[:, :], in_=warm[:, :],
                                 func=mybir.ActivationFunctionType.Exp, scale=LN2)

            # input DMAs: first slot ASAP, then aux, then the rest
            nc.sync.dma_start(out=feat_sb[:, :, 0:1], in_=feat[:, :, 0:1])
            nc.sync.dma_start(out=feat_sb[:, :, 1:2], in_=feat[:, :, 1:2])
            nc.sync.dma_start(out=feat_sb[:, :, 2:5], in_=feat[:, :, 2:5])
            nc.sync.dma_start(out=aux_sb, in_=aux[:, :])
            nc.sync.dma_start(out=feat_sb[:, :, 1:2, NQ:NQ + 4, :],
                              in_=feat[:, :, 1:2, NQ:NQ + 4, :])
            nc.sync.dma_start(out=feat_sb[:, :, 5:11], in_=feat[:, :, 5:11])
            nc.sync.dma_start(out=feat_sb[:, :, 11:NQ], in_=feat[:, :, 11:NQ])

            base_b = aux_sb[:, NQ * 8:NQ * 8 + 1]

            for _rep in range(reps):
                ubank = upool.tile([128, 512], dt.float32, name="ubank")
                first_flip = [True]

                def flips_for(group, e_t):
                    for bi in range(group["b0"], group["b0"] + group["n"]):
                        eoff = (bi - group["b0"]) * BANK
                        for (i, j, cc, boff) in banks[bi]["flips"]:
                            uidx = (i * 4 + cc) * 2
                            last = (j == cc) if i in STRADDLE_SLOTS else (j == 3)
                            nc.tensor.matmul(
                                out=ubank[:, uidx:uidx + 2],
                                lhsT=e_t[:, eoff + boff:eoff + boff + 128],
                                rhs=aux_sb[:, i * 8 + j * 2:i * 8 + j * 2 + 2],
                                start=first_flip[0],
                                stop=last,
                                skip_group_check=True,
                            )
                            first_flip[0] = False

                pending = []
                for gk, group in enumerate(groups):
                    lag = 6 if gk < ng - 1 else 1
                    if gk == ng - 1:
                        for (pq, pe, pw) in dve_q:
                            nc.vector._custom_dve(exp2s, out=pe[:, 0:pw],
                                                  in0=pq[:, 0:pw])
                        dve_q = []
                    while len(pending) > lag:
                        flips_for(*pending.pop(0))
                    b0, n, eng = group["b0"], group["n"], group["eng"]
                    width = n * BANK
                    if eng == "act":
                        g_t = gpa.tile([128, 3 * BANK], dt.float32, name="g_act")
                    else:
                        g_t = gpd.tile([128, BANK], dt.float32, name="g_dve")
                    for bi in range(b0, b0 + n):
                        goff = (bi - b0) * BANK
                        for (i, j, kind, lo, w, off) in banks[bi]["pieces"]:
                            mslot = i if kind == 1 else NQ + STRADDLE_SLOTS.index(i)
                            nc.tensor.matmul(
                                out=g_t[:, goff + off:goff + off + w],
                                lhsT=feat_sb[:, 0, i, :, 128 * j:128 * (j + 1)],
                                rhs=feat_sb[:, 1, mslot, :, lo:lo + w],
                                start=True,
                                stop=True,
                                perf_mode=mybir.MatmulPerfMode.DoubleRow,
                            )
                    e_t = epool.tile([128, 3 * BANK], dt.bfloat16, name="e_t")
                    if eng == "act":
                        nc.scalar.activation(
                            out=e_t[:, 0:width], in_=g_t[:, 0:width],
                            func=mybir.ActivationFunctionType.Exp, scale=LN2)
                    else:
                        nc.vector.scalar_tensor_tensor(
                            out=e_t[:, 0:width],
                            in0=base_b.broadcast_to([128, width]),
                            scalar=1.0,
                            in1=g_t[:, 0:width],
                            op0=mybir.AluOpType.bypass,
                            op1=mybir.AluOpType.pow,
                        )
                    pending.append((group, e_t))
                for p in pending:
                    flips_for(*p)

                res_sb = consts.tile([128, NQ * 8], dt.float32)
                nc.vector.tensor_copy(res_sb[:, :], ubank[:, 0:NQ * 8])
                nc.sync.dma_start(out=out[:, :], in_=res_sb[:, :])

    nc.compile()
    return nc


def _get_program(reps=1):
    if reps not in _PROGRAM_CACHE:
        _PROGRAM_CACHE[reps] = _build_program(reps)
    return _PROGRAM_CACHE[reps]


def _quad_assignment():
    """Per-image quad lists for the two cores sharing an image.
    Straddle quads (c,c) must land on STRADDLE_SLOTS."""
    full = [(c, q) for c in range(8) for q in range(c)]  # 28
    stra = [(c, c) for c in range(8)]                    # 8

    def arrange(fulls, stras):
        fi, si = iter(fulls), iter(stras)
        return [next(si) if s in STRADDLE_SLOTS else next(fi) for s in range(NQ)]

    even = arrange(full[0::2], stra[0:4])
    odd = arrange(full[1::2], stra[4:8])
    return even, odd


def _split_fp8(x, n):
    terms = []
    r = np.asarray(x, np.float64).copy()
    for _ in range(n):
        t = r.astype(_fp8).astype(np.float64)
        terms.append(t)
        r = r - t
    return terms, r


def _prepare_inputs(images, segmentations):
    """Host-side shard/pack. Returns (in_maps, combine_info)."""
    N = images.shape[0]
    assert images.shape == (4, 3, 128, 128) and segmentations.shape == (4, 2, 128, 128)

    img = images[:, :, ::2, ::2].astype(np.float64)  # nearest, [4,3,64,64]

    s = segmentations.astype(np.float32)
    t = s[:, :, 0::2, :] * np.float32(0.5) + s[:, :, 1::2, :] * np.float32(0.5)
    seg = t[:, :, :, 0::2] * np.float32(0.5) + t[:, :, :, 1::2] * np.float32(0.5)
    seg = seg.reshape(N, 2, 4096).astype(np.float64)  # bilinear = 2x2 avg

    sxy = SIGMA_XY * SCALE
    yy, xx = np.meshgrid(np.arange(64.0), np.arange(64.0), indexing="ij")
    pos = np.stack([xx, yy], 0) / sxy
    feats = np.concatenate(
        [np.broadcast_to(pos[None], (N, 2, 64, 64)), img / SIGMA_RGB], axis=1
    ).reshape(N, 5, 4096)
    F = feats - feats.mean(axis=2, keepdims=True)
    F = F * np.sqrt(LOG2E)                       # log2 units
    B = -0.5 * (F * F).sum(axis=1)               # [4,P]

    # per image: 42-row stat/mov/movd stacks + corrected weights
    STAT = np.zeros((N, 42, 4096))
    MOV = np.zeros((N, 42, 4096))
    MOVD = np.zeros((N, 42, 4096))
    W = np.zeros((N, 2, 4096))
    for im in range(N):
        Fs, _ = _split_fp8(F[im], 4)             # Fs[t][5,P], 1-based below
        cs, delta = _split_fp8(B[im], 2)
        W[im] = seg[im] * np.exp2(delta)[None, :]
        r = 0
        for d in range(5):
            for (a, b) in (_PAIRS_XY if d < 2 else _PAIRS_RGB):
                STAT[im, r] = Fs[a - 1][d]
                MOV[im, r] = Fs[b - 1][d]
                r += 1
        for c in cs:                              # stat c rows, moving ones
            STAT[im, r] = c
            MOV[im, r] = 1.0
            r += 1
        for c in cs:                              # stat ones, moving c cols
            STAT[im, r] = 1.0
            MOV[im, r] = c
            r += 1
        STAT[im, r] = 1.0                         # diag-halving bias row
        MOV[im, r] = 0.0
        r += 1
        assert r == 41
        MOVD[im] = MOV[im]
        MOVD[im, 40] = -1.0

    W_bf = W.astype(_bf16)

    even, odd = _quad_assignment()
    in_maps = []
    wcols = []
    for core in range(8):
        im = core // 2
        quads = even if core % 2 == 0 else odd
        feat_arr = np.zeros((KP, 2, 2, NQ + 4, 512), _fp8)
        aux_arr = np.zeros((128, NQ * 8 + 1), _bf16)
        wcol = np.zeros((NQ, 4, 2, 128))
        for slot, (c, q) in enumerate(quads):
            st = STAT[im][:, 512 * q:512 * (q + 1)]
            mv = MOV[im][:, 512 * c:512 * (c + 1)]
            md = MOVD[im][:, 512 * c:512 * (c + 1)]
            feat_arr[:, 0, slot] = st.reshape(KP, 2, 512).astype(_fp8)
            feat_arr[:, 1, slot] = mv.reshape(KP, 2, 512).astype(_fp8)
            if slot in STRADDLE_SLOTS:
                feat_arr[:, :, 1, NQ + STRADDLE_SLOTS.index(slot), :] = \
                    md.reshape(KP, 2, 512).astype(_fp8)
            for j in range(4):
                rlo = 512 * q + 128 * j
                aux_arr[:, slot * 8 + j * 2:slot * 8 + j * 2 + 2] = \
                    W_bf[im][:, rlo:rlo + 128].T
            for cc in range(4):
                clo = 512 * c + 128 * cc
                wcol[slot, cc] = W_bf[im][:, clo:clo + 128].astype(np.float64)
        aux_arr[:, NQ * 8] = _bf16(2.0)
        in_maps.append({"feat": np.ascontiguousarray(feat_arr),
                        "aux": np.ascontiguousarray(aux_arr)})
        wcols.append(wcol)
    return in_maps, wcols


def _combine(outs, wcols, n_images=4):
    total = 0.0
    for core, o in enumerate(outs):
        u = np.asarray(o["out"], np.float64).reshape(128, NQ, 4, 2)
        # sum_m u[m, slot, chunk, k] * wcol[slot, chunk, k, m]
        total += np.einsum("mick,ickm->", u, wcols[core])
    loss = -WEIGHT * 2.0 * total / n_images
    return np.array([loss], dtype=np.float32)


def kernel(images, segmentations):
    from concourse.bass_utils import run_bass_kernel_spmd

    in_maps, wcols = _prepare_inputs(np.asarray(images), np.asarray(segmentations))
    nc = _get_program(reps=1)
    last_err = None
    for _attempt in range(3):  # the NRT backend occasionally fails transiently
        try:
            res = run_bass_kernel_spmd(nc, in_maps, core_ids=list(range(8)))
            return _combine(res.results, wcols)
        except Exception as e:  # noqa: BLE001
            last_err = e
    raise last_err
